# revision 22
# baseline (speedup 1.0000x reference)
"""BERT self-attention Bass kernel for 8 Trainium2 NeuronCores.

Problem: hidden_states [2, 2048, 768], 12 heads x 64 dim, fp32.

Sharding (zero-communication): core c in 0..7 handles batch b = c//4 and
head-group g = c%4 (3 heads). Host pre-lays inputs per core (fp16):
  - hsT   [768, 2048]  hidden[b].T
  - w     [768, 640]   QKV weight columns packed into 5 psum row-groups:
                       g0=[q0|q1] g1=[k0|k1] g2=[q2|v0] g3=[k2|v1] g4=[v2|0]
                       (pairing keeps each head's Q and K partition-aligned;
                       softmax 1/8 folded into Wq)
  - bias  [128, 5]     combined bias per row-group (fp32)
  - maskr [128, 17]    attention_mask[b] column-tiled (col i = keys
                       i*128..i*128+127); col 16 unused
  - ident [128, 128]   identity (PE transposes + PE filler)

Device pipeline per core (fp16 matmuls, fp32 psum accumulate):
  1. QKVT[640, 2048] = w.T @ hsT (d-major), groups in order g2,g3,g4,g0,g1
     so V (and head 2's attention) can start earliest
  2. V transposed back to [t, d] via PE transposes; V_aug[t, 65] per head =
     exp(mask_t) * [V | 1]  (mask folded into V so exp needs no bias and
     the softmax denominator falls out of the PV matmul's ones column)
  3. per (head, s-block of 512), heads in order 2,0,1: for each t-pair:
     2 scores matmuls (K=64) -> one exp over [128,1024] on ACT
     (unnormalized, no max-sub: scores are O(6) by construction) -> 2 PV
     matmuls accumulate ctxT[65, s]; one filler matmul per step keeps the
     PE HAM clock gate at 2.4 GHz (an idle PE is clocked down to 1.2 GHz,
     halving matmul speed - ACT and PE run within ~5% of each other here)
Host: divide rows 0..63 by row 64, transpose to [s, d], interleave heads.
"""

import os

import numpy as np

import concourse.bass as bass
import concourse.mybir as mybir
import concourse.tile as tile
from concourse import bacc
from concourse.bass_utils import run_bass_kernel_spmd

F32 = mybir.dt.float32
F16 = mybir.dt.float16

B = 2
S = 2048
HID = 768
NH = 12          # total heads
D = 64           # head dim
NHL = 3          # heads per core
DG = NHL * D     # 192 cols of each W per core
QKV = 640        # packed QKVT row space (5 groups of 128)
KC = HID // 128  # 6 contraction chunks
NG = 5           # psum row-groups of QKVT
GW = [128, 128, 128, 128, 64]     # real rows per group
NT = S // 128    # 16 key tiles
SBW = 512        # s-block width
NSB = S // SBW   # 4 s-blocks

# (group, offset) per quantity and head
QPOS = [(0, 0), (0, 64), (2, 0)]
KPOS = [(1, 0), (1, 64), (3, 0)]
VPOS = [(2, 64), (3, 64), (4, 0)]
HORDER = [2, 0, 1]  # head 2's tiles are ready first

LAST_EXEC_TIME_NS = None

_CACHED_NC = None


def _build_nc():
    nc = bacc.Bacc("TRN2", target_bir_lowering=False, debug=False, num_devices=8)

    hsT_d = nc.dram_tensor("hsT", [HID, S], F16, kind="ExternalInput")
    w_d = nc.dram_tensor("w", [HID, QKV], F16, kind="ExternalInput")
    bias_d = nc.dram_tensor("bias", [128, NG], F32, kind="ExternalInput")
    maskr_d = nc.dram_tensor("maskr", [128, NT + 1], F32, kind="ExternalInput")
    ident_d = nc.dram_tensor("ident", [128, 128], F16, kind="ExternalInput")
    out_d = nc.dram_tensor("ctxa", [NHL, D + 1, S], F32, kind="ExternalOutput")

    with tile.TileContext(nc) as tc:
        with (
            tc.tile_pool(name="const", bufs=1) as cp,
            tc.tile_pool(name="qkvt", bufs=1) as qp,
            tc.tile_pool(name="vaug", bufs=1) as vp,
            tc.tile_pool(name="probs", bufs=3) as pp,
            tc.tile_pool(name="oc", bufs=3) as op,
            tc.tile_pool(name="ps_a", bufs=2, space="PSUM") as ps_a,
            tc.tile_pool(name="ps_b", bufs=2, space="PSUM") as ps_b,
            tc.tile_pool(name="ps_c", bufs=2, space="PSUM") as ps_c,
        ):
            # --- constants / inputs ---
            # hs chunks stream on the SP HWDGE ring; weights and small
            # tensors on the ACT ring so descriptor generation for both
            # proceeds in parallel and the first QKV matmul starts early.
            ident = cp.tile([128, 128], F16, tag="ident")
            nc.scalar.dma_start(ident[:], ident_d.ap())
            w_sb = cp.tile([128, KC, QKV], F16, tag="w")
            w_ap = w_d.ap().rearrange("(kc p) n -> p kc n", p=128)
            # warm-filler source first in the gpsimd queue: the PE warm-up
            # must not wait behind SWDGE descriptor generation
            wfsrc = cp.tile([128, 16], F16, tag="wfsrc")
            nc.gpsimd.memset(wfsrc[:], 1.0)
            hs = []
            for kc in range(KC):
                t = cp.tile([128, S], F16, tag=f"hsT{kc}", name=f"hsT{kc}")
                eng = nc.sync if kc % 2 == 0 else nc.gpsimd
                eng.dma_start(t[:], hsT_d.ap()[kc * 128 : (kc + 1) * 128, :])
                nc.scalar.dma_start(
                    w_sb[:, kc : kc + 1, :], w_ap[:, kc : kc + 1, :]
                )
                hs.append(t)
            bias_sb = cp.tile([128, NG], F32, tag="bias")
            nc.scalar.dma_start(bias_sb[:], bias_d.ap())
            maskr_sb = cp.tile([128, NT + 1], F32, tag="maskr")
            nc.scalar.dma_start(maskr_sb[:], maskr_d.ap())

            # em[t] = exp(mask_t), folded into V_aug below
            em = cp.tile([128, NT], F32, tag="em")
            nc.scalar.activation(
                em[:], maskr_sb[:, :NT], mybir.ActivationFunctionType.Exp
            )

            # --- QKVT = w.T @ hsT (d-major) ---
            qkvt = [
                qp.tile([128, S], F16, tag=f"qkvt{g}", name=f"qkvt{g}")
                for g in range(NG)
            ]
            # K weights per head in [128, S] tiles with the other 64
            # partitions zeroed: scores matmuls then contract over K=128,
            # which enables the fast weight load (the zero rows multiply
            # whatever sits in the rhs partitions and contribute nothing)
            ktp = [
                qp.tile([128, S], F16, tag=f"ktp{h}", name=f"ktp{h}")
                for h in range(NHL)
            ]
            nc.gpsimd.memset(ktp[0][64:128, :], 0.0)
            nc.gpsimd.memset(ktp[1][0:64, :], 0.0)
            nc.gpsimd.memset(ktp[2][64:128, :], 0.0)

            def qkv_units(gi, pool=None, tag="acc", nsb=1):
                """Yield one closure per matmul of group gi; each s-block
                run ends with its psum->sbuf copy. nsb = s-blocks per
                psum accumulator tile."""
                gw = GW[gi]
                pool = pool or ps_c
                for sb0 in range(0, NSB, nsb):
                    acc = pool.tile(
                        [128, nsb * SBW], F32, tag=tag, name="acc"
                    )
                    for kc in range(KC):
                        for i in range(nsb):
                            s0 = (sb0 + i) * SBW

                            def mm(kc=kc, i=i, acc=acc, s0=s0, gw=gw, gi=gi):
                                nc.tensor.matmul(
                                    acc[:gw, i * SBW : (i + 1) * SBW],
                                    w_sb[:, kc, gi * 128 : gi * 128 + gw],
                                    hs[kc][:, s0 : s0 + SBW],
                                    start=(kc == 0),
                                    stop=(kc == KC - 1),
                                )
                                if kc == KC - 1:
                                    a = acc[:, i * SBW : (i + 1) * SBW]
                                    sl = slice(s0, s0 + SBW)

                                    def cp(dst, rows, b0):
                                        nc.vector.tensor_scalar_add(
                                            dst[rows, sl],
                                            a[rows, :],
                                            bias_sb[
                                                b0 : b0
                                                + (rows.stop - rows.start),
                                                gi : gi + 1,
                                            ],
                                        )

                                    lo, hi = slice(0, 64), slice(64, 128)
                                    if gi == 0:  # q0|q1
                                        cp(qkvt[0], slice(0, 128), 0)
                                    elif gi == 1:  # k0|k1 -> ktp
                                        cp(ktp[0], lo, 0)
                                        cp(ktp[1], hi, 64)
                                    elif gi == 2:  # q2|v0
                                        cp(qkvt[2], slice(0, 128), 0)
                                    elif gi == 3:  # k2|v1
                                        cp(ktp[2], lo, 0)
                                        cp(qkvt[3], hi, 64)
                                    else:  # v2
                                        cp(qkvt[4], lo, 0)

                            yield mm

            def qkv_group(gi, pool=None, tag="acc", nsb=1, warm=0):
                """warm: filler matmuls emitted before each real matmul to
                keep the PE busy (and the HAM clock warming) while the hsT
                chunks are still streaming in."""
                for u in qkv_units(gi, pool, tag, nsb):
                    for _ in range(warm):
                        wf = ps_c.tile([128, SBW], F32, tag="acc", name="wf")
                        nc.tensor.matmul(
                            wf[:, :128],
                            ident[:],
                            ident[:],
                            start=True,
                            stop=True,
                            skip_group_check=True,
                        )
                    u()

            def make_vaug_half(h, vg, half):
                ti, off = VPOS[h]
                vt = qkvt[ti]
                tp = ps_b.tile([128, 8 * D], F16, tag="ps", name="tp")
                for j in range(8):
                    tt = half * 8 + j
                    nc.tensor.transpose(
                        tp[:, j * D : (j + 1) * D],
                        vt[off : off + D, tt * 128 : (tt + 1) * 128],
                        ident[off : off + D, off : off + D],
                    )
                nc.vector.tensor_tensor(
                    vg[:, half * 8 : (half + 1) * 8, :D],
                    tp[:].rearrange("p (j d) -> p j d", d=D),
                    em[:, half * 8 : (half + 1) * 8]
                    .rearrange("p (j o) -> p j o", o=1)
                    .broadcast_to([128, 8, D]),
                    mybir.AluOpType.mult,
                )
                if half == 0:
                    nc.vector.tensor_copy(
                        vg[:, :, D : D + 1],
                        em[:, :NT].rearrange("p (j o) -> p j o", o=1),
                    )

            def make_vaug(h, vg):
                make_vaug_half(h, vg, 0)
                make_vaug_half(h, vg, 1)

            vaug = [
                vp.tile([128, NT, D + 1], F16, tag=f"vaug{h}", name=f"vaug{h}")
                for h in range(NHL)
            ]

            # PE warm-up with no DMA dependency: tiny matmuls on the
            # memset tile keep the PE busy from engine-init onward so the
            # HAM clock gate is warming before the first hsT chunk lands.
            # Tiles are pre-allocated so the psum slot rotation matches
            # execution order (allocating them between the QKV accumulators
            # would make fillers wait on attention-time releases).
            wf_tiles = iter(
                [
                    ps_c.tile([128, SBW], F32, tag="acc", name=f"wf{i}")
                    for i in range(64)
                ]
            )

            def warm(n):
                for _ in range(n):
                    wf = next(wf_tiles, None)
                    if wf is None:
                        return
                    nc.tensor.matmul(
                        wf[:16, :16],
                        wfsrc[:],
                        wfsrc[:],
                        start=True,
                        stop=True,
                        skip_group_check=True,
                    )

            warm(30)

            # Phase 1 runs kc-major across six live psum accumulators so the
            # PE's static instruction order matches the hsT chunk arrival
            # order (group-major would head-of-line block on chunk 5);
            # fillers per kc round bridge the chunk-arrival stalls so a
            # 3.4us continuous-busy window can complete and warm the clock.
            g2u = list(qkv_units(2, ps_a, "ps", nsb=2))   # 2 accs x 12
            g4u = list(qkv_units(4, ps_c, "acc", nsb=1))  # 4 accs x 6
            g3u = list(qkv_units(3, ps_b, "ps", nsb=1))   # 4 accs x 6
            NWARM = [0, 2, 4, 8, 10, 10]
            for kc in range(KC):
                warm(NWARM[kc])
                for a in range(2):
                    g2u[a * 2 * KC + kc * 2]()
                    g2u[a * 2 * KC + kc * 2 + 1]()
                for a in range(2):
                    g4u[a * KC + kc]()
                for a in range(2):
                    g3u[a * KC + kc]()
            make_vaug_half(2, vaug[2], 0)

            # --- attention ---
            # Remaining QKV work runs inside head 2's attention in place of
            # fillers: (h2, sbk0)'s first steps absorb the leftover g3/g4
            # s-blocks and vaug halves; the rest of h2 absorbs g0/g1 (q0 q1
            # k0 k1) and the h0/h1 V transposes. The PE never idles long, so
            # the HAM clock stays at 2.4 GHz.
            def vaug_units(h):
                def unit(h=h):
                    make_vaug(h, vaug[h])

                yield unit

            early = list(g3u[2 * KC :])
            early.extend(g4u[2 * KC :])
            early.append(lambda: make_vaug_half(2, vaug[2], 1))
            interleave = [u for gi in (0, 1) for u in qkv_units(gi)]
            interleave[24:24] = list(vaug_units(0))
            interleave.append(next(vaug_units(1)))
            ipos = 0
            epos = 0
            gstep = 0
            NSTEP = NT // 2
            h2_steps = NSB * NSTEP - NSTEP  # main interleave: h2 minus sbk0

            # software-pipelined emission: the PV pair of step k is emitted
            # AFTER exp(k+1), so in the PE's static order the next scores
            # pair runs while ACT is busy and ACT never waits on the PE.
            pending = [None]  # (h, ctx, st, pr, oc_args) awaiting PV emission

            def flush_pending():
                if pending[0] is None:
                    return
                (ph, pctx, pst, ppr, poc) = pending[0]
                for half in range(2):
                    tt = pst * 2 + half
                    nc.tensor.matmul(
                        pctx[: D + 1, :],
                        vaug[ph][:, tt, :],
                        ppr[:, half * SBW : (half + 1) * SBW],
                        start=(tt == 0),
                        stop=(tt == NT - 1),
                    )
                if poc is not None:
                    h_, s0_ = poc
                    oc = op.tile([128, SBW], F32, tag="oc", name="oc")
                    nc.vector.tensor_copy(oc[: D + 1, :], pctx[: D + 1, :])
                    nc.sync.dma_start(
                        out_d.ap()[h_, :, s0_ : s0_ + SBW],
                        oc[: D + 1, :],
                    )

            QTILE = [0, 0, 2]  # rhs tile per head (full 128 partitions)
            for h in HORDER:
                qt, kt = qkvt[QTILE[h]], ktp[h]
                for sbk in range(NSB):
                    s0 = sbk * SBW
                    ctx = ps_b.tile([128, SBW], F32, tag="ps", name="ctx")
                    for st in range(NSTEP):  # t-pair steps
                        sc = ps_a.tile([128, 2 * SBW], F32, tag="ps", name="sc")
                        for half in range(2):
                            tt = st * 2 + half
                            nc.tensor.matmul(
                                sc[:, half * SBW : (half + 1) * SBW],
                                kt[:, tt * 128 : (tt + 1) * 128],
                                qt[:, s0 : s0 + SBW],
                                start=True,
                                stop=True,
                            )
                        if gstep < NSTEP:
                            # (h2, sbk0): drain the early queue, 6 per step
                            want_e = (gstep + 1) * 6
                            while epos < min(want_e, len(early)):
                                early[epos]()
                                epos += 1
                        else:
                            gs = gstep - NSTEP
                            want = (
                                (gs + 1) * len(interleave) + h2_steps - 1
                            ) // h2_steps
                            while ipos < min(want, len(interleave)):
                                interleave[ipos]()
                                ipos += 1
                        pr = pp.tile([128, 2 * SBW], F16, tag="pr", name="pr")
                        nc.scalar.activation(
                            pr[:], sc[:], mybir.ActivationFunctionType.Exp
                        )
                        flush_pending()
                        pending[0] = (
                            h,
                            ctx,
                            st,
                            pr,
                            (h, s0) if st == NSTEP - 1 else None,
                        )
                        gstep += 1
            flush_pending()

    nc.compile()
    return nc


def _get_nc():
    global _CACHED_NC
    if _CACHED_NC is None:
        _CACHED_NC = _build_nc()
    return _CACHED_NC


def kernel(
    hidden_states, attention_mask, Wq, bq, Wk, bk, Wv, bv
) -> np.ndarray:
    global LAST_EXEC_TIME_NS
    hidden_states = np.asarray(hidden_states, dtype=np.float32)
    attention_mask = np.asarray(attention_mask, dtype=np.float32)
    Wq = np.asarray(Wq, dtype=np.float32)
    Wk = np.asarray(Wk, dtype=np.float32)
    Wv = np.asarray(Wv, dtype=np.float32)
    bq = np.asarray(bq, dtype=np.float32)
    bk = np.asarray(bk, dtype=np.float32)
    bv = np.asarray(bv, dtype=np.float32)

    scale = 1.0 / np.sqrt(np.float32(D))

    in_maps = []
    for c in range(8):
        b, g = divmod(c, 4)
        cols = slice(g * DG, (g + 1) * DG)
        wq = Wq[:, cols] * scale
        wk = Wk[:, cols]
        wv = Wv[:, cols]
        w = np.zeros((HID, QKV), dtype=np.float32)
        bcat = np.zeros(QKV, dtype=np.float32)
        bq_, bk_, bv_ = bq[cols] * scale, bk[cols], bv[cols]
        for h in range(NHL):
            for (pos, mat, bb) in (
                (QPOS[h], wq, bq_),
                (KPOS[h], wk, bk_),
                (VPOS[h], wv, bv_),
            ):
                gi, off = pos
                r0 = gi * 128 + off
                w[:, r0 : r0 + D] = mat[:, h * D : (h + 1) * D]
                bcat[r0 : r0 + D] = bb[h * D : (h + 1) * D]
        bias = np.ascontiguousarray(bcat.reshape(NG, 128).T)
        maskr = np.zeros((128, NT + 1), dtype=np.float32)
        maskr[:, :NT] = attention_mask[b, 0, 0, :].reshape(NT, 128).T
        in_maps.append(
            {
                "hsT": np.ascontiguousarray(hidden_states[b].T).astype(np.float16),
                "w": w.astype(np.float16),
                "bias": bias,
                "maskr": maskr,
                "ident": np.eye(128, dtype=np.float16),
            }
        )

    nc = _get_nc()
    trace = bool(os.environ.get("BASS_KERNEL_TRACE"))
    res = run_bass_kernel_spmd(nc, in_maps, list(range(8)), trace=trace)
    LAST_EXEC_TIME_NS = res.exec_time_ns

    out = np.empty((B, S, HID), dtype=np.float32)
    for c in range(8):
        b, g = divmod(c, 4)
        ctxa = res.results[c]["ctxa"]  # [3, 65, 2048]
        for hl in range(NHL):
            ctx = ctxa[hl, :D, :] / ctxa[hl, D : D + 1, :]  # [64, 2048]
            out[b, :, g * DG + hl * D : g * DG + (hl + 1) * D] = ctx.T
    return out


# revision 23
# speedup vs baseline: 1.0022x; 1.0022x over previous
"""BERT self-attention Bass kernel for 8 Trainium2 NeuronCores.

Problem: hidden_states [2, 2048, 768], 12 heads x 64 dim, fp32.

Sharding (zero-communication): core c in 0..7 handles batch b = c//4 and
head-group g = c%4 (3 heads). Host pre-lays inputs per core (fp16):
  - hsT   [768, 2048]  hidden[b].T
  - w     [768, 640]   QKV weight columns packed into 5 psum row-groups:
                       g0=[q0|q1] g1=[k0|k1] g2=[q2|v0] g3=[k2|v1] g4=[v2|0]
                       (pairing keeps each head's Q and K partition-aligned;
                       softmax 1/8 folded into Wq)
  - bias  [128, 5]     combined bias per row-group (fp32)
  - maskr [128, 17]    attention_mask[b] column-tiled (col i = keys
                       i*128..i*128+127); col 16 unused
  - ident [128, 128]   identity (PE transposes + PE filler)

Device pipeline per core (fp16 matmuls, fp32 psum accumulate):
  1. QKVT[640, 2048] = w.T @ hsT (d-major), groups in order g2,g3,g4,g0,g1
     so V (and head 2's attention) can start earliest
  2. V transposed back to [t, d] via PE transposes; V_aug[t, 65] per head =
     exp(mask_t) * [V | 1]  (mask folded into V so exp needs no bias and
     the softmax denominator falls out of the PV matmul's ones column)
  3. per (head, s-block of 512), heads in order 2,0,1: for each t-pair:
     2 scores matmuls (K=64) -> one exp over [128,1024] on ACT
     (unnormalized, no max-sub: scores are O(6) by construction) -> 2 PV
     matmuls accumulate ctxT[65, s]; one filler matmul per step keeps the
     PE HAM clock gate at 2.4 GHz (an idle PE is clocked down to 1.2 GHz,
     halving matmul speed - ACT and PE run within ~5% of each other here)
Host: divide rows 0..63 by row 64, transpose to [s, d], interleave heads.
"""

import os

import numpy as np

import concourse.bass as bass
import concourse.mybir as mybir
import concourse.tile as tile
from concourse import bacc
from concourse.bass_utils import run_bass_kernel_spmd

F32 = mybir.dt.float32
F16 = mybir.dt.float16

B = 2
S = 2048
HID = 768
NH = 12          # total heads
D = 64           # head dim
NHL = 3          # heads per core
DG = NHL * D     # 192 cols of each W per core
QKV = 640        # packed QKVT row space (5 groups of 128)
KC = HID // 128  # 6 contraction chunks
NG = 5           # psum row-groups of QKVT
GW = [128, 128, 128, 128, 64]     # real rows per group
NT = S // 128    # 16 key tiles
SBW = 512        # s-block width
NSB = S // SBW   # 4 s-blocks

# (group, offset) per quantity and head
QPOS = [(0, 0), (0, 64), (2, 0)]
KPOS = [(1, 0), (1, 64), (3, 0)]
VPOS = [(2, 64), (3, 64), (4, 0)]
HORDER = [2, 0, 1]  # head 2's tiles are ready first

LAST_EXEC_TIME_NS = None

_CACHED_NC = None


def _build_nc():
    nc = bacc.Bacc("TRN2", target_bir_lowering=False, debug=False, num_devices=8)

    hsT_d = nc.dram_tensor("hsT", [HID, S], F16, kind="ExternalInput")
    w_d = nc.dram_tensor("w", [HID, QKV], F16, kind="ExternalInput")
    bias_d = nc.dram_tensor("bias", [128, NG], F32, kind="ExternalInput")
    maskr_d = nc.dram_tensor("maskr", [128, NT + 1], F32, kind="ExternalInput")
    ident_d = nc.dram_tensor("ident", [128, 128], F16, kind="ExternalInput")
    out_d = nc.dram_tensor("ctxa", [NHL, D + 1, S], F32, kind="ExternalOutput")

    with tile.TileContext(nc) as tc:
        with (
            tc.tile_pool(name="const", bufs=1) as cp,
            tc.tile_pool(name="qkvt", bufs=1) as qp,
            tc.tile_pool(name="vaug", bufs=1) as vp,
            tc.tile_pool(name="probs", bufs=3) as pp,
            tc.tile_pool(name="oc", bufs=3) as op,
            tc.tile_pool(name="ps_a", bufs=2, space="PSUM") as ps_a,
            tc.tile_pool(name="ps_b", bufs=2, space="PSUM") as ps_b,
            tc.tile_pool(name="ps_c", bufs=2, space="PSUM") as ps_c,
        ):
            # --- constants / inputs ---
            # hs chunks stream on the SP HWDGE ring; weights and small
            # tensors on the ACT ring so descriptor generation for both
            # proceeds in parallel and the first QKV matmul starts early.
            ident = cp.tile([128, 128], F16, tag="ident")
            nc.scalar.dma_start(ident[:], ident_d.ap())
            w_sb = cp.tile([128, KC, QKV], F16, tag="w")
            w_ap = w_d.ap().rearrange("(kc p) n -> p kc n", p=128)
            # warm-filler source first in the gpsimd queue: the PE warm-up
            # must not wait behind SWDGE descriptor generation
            wfsrc = cp.tile([128, 16], F16, tag="wfsrc")
            nc.gpsimd.memset(wfsrc[:], 1.0)
            hs = []
            for kc in range(KC):
                t = cp.tile([128, S], F16, tag=f"hsT{kc}", name=f"hsT{kc}")
                eng = nc.sync if kc % 2 == 0 else nc.gpsimd
                eng.dma_start(t[:], hsT_d.ap()[kc * 128 : (kc + 1) * 128, :])
                nc.scalar.dma_start(
                    w_sb[:, kc : kc + 1, :], w_ap[:, kc : kc + 1, :]
                )
                hs.append(t)
            bias_sb = cp.tile([128, NG], F32, tag="bias")
            nc.scalar.dma_start(bias_sb[:], bias_d.ap())
            maskr_sb = cp.tile([128, NT + 1], F32, tag="maskr")
            nc.scalar.dma_start(maskr_sb[:], maskr_d.ap())

            # em[t] = exp(mask_t), folded into V_aug below
            em = cp.tile([128, NT], F32, tag="em")
            nc.scalar.activation(
                em[:], maskr_sb[:, :NT], mybir.ActivationFunctionType.Exp
            )

            # --- QKVT = w.T @ hsT (d-major) ---
            qkvt = [
                qp.tile([128, S], F16, tag=f"qkvt{g}", name=f"qkvt{g}")
                for g in range(NG)
            ]
            # K weights per head in [128, S] tiles with the other 64
            # partitions zeroed: scores matmuls then contract over K=128,
            # which enables the fast weight load (the zero rows multiply
            # whatever sits in the rhs partitions and contribute nothing)
            ktp = [
                qp.tile([128, S], F16, tag=f"ktp{h}", name=f"ktp{h}")
                for h in range(NHL)
            ]
            nc.gpsimd.memset(ktp[0][64:128, :], 0.0)
            nc.gpsimd.memset(ktp[1][0:64, :], 0.0)
            nc.gpsimd.memset(ktp[2][64:128, :], 0.0)

            def qkv_units(gi, pool=None, tag="acc", nsb=1):
                """Yield one closure per matmul of group gi; each s-block
                run ends with its psum->sbuf copy. nsb = s-blocks per
                psum accumulator tile."""
                gw = GW[gi]
                pool = pool or ps_c
                for sb0 in range(0, NSB, nsb):
                    acc = pool.tile(
                        [128, nsb * SBW], F32, tag=tag, name="acc"
                    )
                    for kc in range(KC):
                        for i in range(nsb):
                            s0 = (sb0 + i) * SBW

                            def mm(kc=kc, i=i, acc=acc, s0=s0, gw=gw, gi=gi):
                                nc.tensor.matmul(
                                    acc[:gw, i * SBW : (i + 1) * SBW],
                                    w_sb[:, kc, gi * 128 : gi * 128 + gw],
                                    hs[kc][:, s0 : s0 + SBW],
                                    start=(kc == 0),
                                    stop=(kc == KC - 1),
                                )
                                if kc == KC - 1:
                                    a = acc[:, i * SBW : (i + 1) * SBW]
                                    sl = slice(s0, s0 + SBW)

                                    def cp(dst, rows, b0):
                                        nc.vector.tensor_scalar_add(
                                            dst[rows, sl],
                                            a[rows, :],
                                            bias_sb[
                                                b0 : b0
                                                + (rows.stop - rows.start),
                                                gi : gi + 1,
                                            ],
                                        )

                                    lo, hi = slice(0, 64), slice(64, 128)
                                    if gi == 0:  # q0|q1
                                        cp(qkvt[0], slice(0, 128), 0)
                                    elif gi == 1:  # k0|k1 -> ktp
                                        cp(ktp[0], lo, 0)
                                        cp(ktp[1], hi, 64)
                                    elif gi == 2:  # q2|v0
                                        cp(qkvt[2], slice(0, 128), 0)
                                    elif gi == 3:  # k2|v1
                                        cp(ktp[2], lo, 0)
                                        cp(qkvt[3], hi, 64)
                                    else:  # v2
                                        cp(qkvt[4], lo, 0)

                            yield mm

            def qkv_group(gi, pool=None, tag="acc", nsb=1, warm=0):
                """warm: filler matmuls emitted before each real matmul to
                keep the PE busy (and the HAM clock warming) while the hsT
                chunks are still streaming in."""
                for u in qkv_units(gi, pool, tag, nsb):
                    for _ in range(warm):
                        wf = ps_c.tile([128, SBW], F32, tag="acc", name="wf")
                        nc.tensor.matmul(
                            wf[:, :128],
                            ident[:],
                            ident[:],
                            start=True,
                            stop=True,
                            skip_group_check=True,
                        )
                    u()

            def make_vaug_half(h, vg, half):
                ti, off = VPOS[h]
                vt = qkvt[ti]
                tp = ps_b.tile([128, 8 * D], F16, tag="ps", name="tp")
                for j in range(8):
                    tt = half * 8 + j
                    nc.tensor.transpose(
                        tp[:, j * D : (j + 1) * D],
                        vt[off : off + D, tt * 128 : (tt + 1) * 128],
                        ident[off : off + D, off : off + D],
                    )
                nc.vector.tensor_tensor(
                    vg[:, half * 8 : (half + 1) * 8, :D],
                    tp[:].rearrange("p (j d) -> p j d", d=D),
                    em[:, half * 8 : (half + 1) * 8]
                    .rearrange("p (j o) -> p j o", o=1)
                    .broadcast_to([128, 8, D]),
                    mybir.AluOpType.mult,
                )
                if half == 0:
                    nc.vector.tensor_copy(
                        vg[:, :, D : D + 1],
                        em[:, :NT].rearrange("p (j o) -> p j o", o=1),
                    )

            def make_vaug(h, vg):
                make_vaug_half(h, vg, 0)
                make_vaug_half(h, vg, 1)

            vaug = [
                vp.tile([128, NT, D + 1], F16, tag=f"vaug{h}", name=f"vaug{h}")
                for h in range(NHL)
            ]

            # PE warm-up with no DMA dependency: tiny matmuls on the
            # memset tile keep the PE busy from engine-init onward so the
            # HAM clock gate is warming before the first hsT chunk lands.
            # Tiles are pre-allocated so the psum slot rotation matches
            # execution order (allocating them between the QKV accumulators
            # would make fillers wait on attention-time releases).
            wf_tiles = iter(
                [
                    ps_c.tile([128, SBW], F32, tag="acc", name=f"wf{i}")
                    for i in range(64)
                ]
            )

            def warm(n):
                for _ in range(n):
                    wf = next(wf_tiles, None)
                    if wf is None:
                        return
                    nc.tensor.matmul(
                        wf[:16, :16],
                        wfsrc[:],
                        wfsrc[:],
                        start=True,
                        stop=True,
                        skip_group_check=True,
                    )

            warm(30)

            # Phase 1 runs kc-major across six live psum accumulators so the
            # PE's static instruction order matches the hsT chunk arrival
            # order (group-major would head-of-line block on chunk 5);
            # fillers per kc round bridge the chunk-arrival stalls so a
            # 3.4us continuous-busy window can complete and warm the clock.
            g2u = list(qkv_units(2, ps_a, "ps", nsb=2))   # 2 accs x 12
            g4u = list(qkv_units(4, ps_c, "acc", nsb=1))  # 4 accs x 6
            g3u = list(qkv_units(3, ps_b, "ps", nsb=1))   # 4 accs x 6
            NWARM = [0, 2, 4, 8, 10, 10]
            for kc in range(KC):
                warm(NWARM[kc])
                for a in range(2):
                    g2u[a * 2 * KC + kc * 2]()
                    g2u[a * 2 * KC + kc * 2 + 1]()
                for a in range(2):
                    g4u[a * KC + kc]()
                for a in range(2):
                    g3u[a * KC + kc]()
            make_vaug_half(2, vaug[2], 0)
            for u in g3u[2 * KC :]:
                u()
            for u in g4u[2 * KC :]:
                u()
            make_vaug_half(2, vaug[2], 1)

            # --- attention ---
            # Remaining QKV work runs inside head 2's attention in place of
            # fillers: (h2, sbk0)'s first steps absorb the leftover g3/g4
            # s-blocks and vaug halves; the rest of h2 absorbs g0/g1 (q0 q1
            # k0 k1) and the h0/h1 V transposes. The PE never idles long, so
            # the HAM clock stays at 2.4 GHz.
            def vaug_units(h):
                def unit(h=h):
                    make_vaug(h, vaug[h])

                yield unit

            interleave = [u for gi in (0, 1) for u in qkv_units(gi)]
            interleave[24:24] = list(vaug_units(0))
            interleave.append(next(vaug_units(1)))
            ipos = 0
            gstep = 0
            NSTEP = NT // 2
            h2_steps = NSB * NSTEP  # spread interleave over head 2's steps

            # software-pipelined emission: the PV pair of step k is emitted
            # AFTER exp(k+1), so in the PE's static order the next scores
            # pair runs while ACT is busy and ACT never waits on the PE.
            pending = [None]  # (h, ctx, st, pr, oc_args) awaiting PV emission

            def flush_pending():
                if pending[0] is None:
                    return
                (ph, pctx, pst, ppr, poc) = pending[0]
                for half in range(2):
                    tt = pst * 2 + half
                    nc.tensor.matmul(
                        pctx[: D + 1, :],
                        vaug[ph][:, tt, :],
                        ppr[:, half * SBW : (half + 1) * SBW],
                        start=(tt == 0),
                        stop=(tt == NT - 1),
                    )
                if poc is not None:
                    h_, s0_ = poc
                    oc = op.tile([128, SBW], F32, tag="oc", name="oc")
                    nc.vector.tensor_copy(oc[: D + 1, :], pctx[: D + 1, :])
                    nc.sync.dma_start(
                        out_d.ap()[h_, :, s0_ : s0_ + SBW],
                        oc[: D + 1, :],
                    )

            QTILE = [0, 0, 2]  # rhs tile per head (full 128 partitions)
            for h in HORDER:
                qt, kt = qkvt[QTILE[h]], ktp[h]
                for sbk in range(NSB):
                    s0 = sbk * SBW
                    ctx = ps_b.tile([128, SBW], F32, tag="ps", name="ctx")
                    for st in range(NSTEP):  # t-pair steps
                        sc = ps_a.tile([128, 2 * SBW], F32, tag="ps", name="sc")
                        for half in range(2):
                            tt = st * 2 + half
                            nc.tensor.matmul(
                                sc[:, half * SBW : (half + 1) * SBW],
                                kt[:, tt * 128 : (tt + 1) * 128],
                                qt[:, s0 : s0 + SBW],
                                start=True,
                                stop=True,
                            )
                        want = (
                            (gstep + 1) * len(interleave) + h2_steps - 1
                        ) // h2_steps
                        while ipos < min(want, len(interleave)):
                            interleave[ipos]()
                            ipos += 1
                        pr = pp.tile([128, 2 * SBW], F16, tag="pr", name="pr")
                        nc.scalar.activation(
                            pr[:], sc[:], mybir.ActivationFunctionType.Exp
                        )
                        flush_pending()
                        pending[0] = (
                            h,
                            ctx,
                            st,
                            pr,
                            (h, s0) if st == NSTEP - 1 else None,
                        )
                        gstep += 1
            flush_pending()

    nc.compile()
    return nc


def _get_nc():
    global _CACHED_NC
    if _CACHED_NC is None:
        _CACHED_NC = _build_nc()
    return _CACHED_NC


def kernel(
    hidden_states, attention_mask, Wq, bq, Wk, bk, Wv, bv
) -> np.ndarray:
    global LAST_EXEC_TIME_NS
    hidden_states = np.asarray(hidden_states, dtype=np.float32)
    attention_mask = np.asarray(attention_mask, dtype=np.float32)
    Wq = np.asarray(Wq, dtype=np.float32)
    Wk = np.asarray(Wk, dtype=np.float32)
    Wv = np.asarray(Wv, dtype=np.float32)
    bq = np.asarray(bq, dtype=np.float32)
    bk = np.asarray(bk, dtype=np.float32)
    bv = np.asarray(bv, dtype=np.float32)

    scale = 1.0 / np.sqrt(np.float32(D))

    in_maps = []
    for c in range(8):
        b, g = divmod(c, 4)
        cols = slice(g * DG, (g + 1) * DG)
        wq = Wq[:, cols] * scale
        wk = Wk[:, cols]
        wv = Wv[:, cols]
        w = np.zeros((HID, QKV), dtype=np.float32)
        bcat = np.zeros(QKV, dtype=np.float32)
        bq_, bk_, bv_ = bq[cols] * scale, bk[cols], bv[cols]
        for h in range(NHL):
            for (pos, mat, bb) in (
                (QPOS[h], wq, bq_),
                (KPOS[h], wk, bk_),
                (VPOS[h], wv, bv_),
            ):
                gi, off = pos
                r0 = gi * 128 + off
                w[:, r0 : r0 + D] = mat[:, h * D : (h + 1) * D]
                bcat[r0 : r0 + D] = bb[h * D : (h + 1) * D]
        bias = np.ascontiguousarray(bcat.reshape(NG, 128).T)
        maskr = np.zeros((128, NT + 1), dtype=np.float32)
        maskr[:, :NT] = attention_mask[b, 0, 0, :].reshape(NT, 128).T
        in_maps.append(
            {
                "hsT": np.ascontiguousarray(hidden_states[b].T).astype(np.float16),
                "w": w.astype(np.float16),
                "bias": bias,
                "maskr": maskr,
                "ident": np.eye(128, dtype=np.float16),
            }
        )

    nc = _get_nc()
    trace = bool(os.environ.get("BASS_KERNEL_TRACE"))
    res = run_bass_kernel_spmd(nc, in_maps, list(range(8)), trace=trace)
    LAST_EXEC_TIME_NS = res.exec_time_ns

    out = np.empty((B, S, HID), dtype=np.float32)
    for c in range(8):
        b, g = divmod(c, 4)
        ctxa = res.results[c]["ctxa"]  # [3, 65, 2048]
        for hl in range(NHL):
            ctx = ctxa[hl, :D, :] / ctxa[hl, D : D + 1, :]  # [64, 2048]
            out[b, :, g * DG + hl * D : g * DG + (hl + 1) * D] = ctx.T
    return out
